# revision 27
# baseline (speedup 1.0000x reference)
"""Multi-head causal attention with RoPE on 8 Trainium2 NeuronCores.

Sharding: tensor-parallel over heads. Each of the 8 cores owns 2 of the 16
heads (a 256-row slice of w_q/w_k/w_v and the matching 256-column slice of
w_o). x is replicated. Each core computes its partial output projection
outT_c = w_o_slice.T @ ctx_slice in transposed [d, s] layout; the host sums
the 8 partials and transposes back.

On-device layout strategy (per core):
  - x arrives transposed: xT [2048, 4096] so projections contract d on
    partitions. q/k are produced in [head_dim, s] layout (RoPE applied via
    partition-half swizzle on DVE); v in natural [s, head_dim] layout.
  - attention runs in "scoresT" geometry: scoresT[k, q] = kT_tile.T @ qT,
    exp on ScalarE over k-tile PAIRS (scale 1/sqrt(128) fused, no max
    subtraction - scaled scores are bounded ~|6.5| so fp32 exp is safe),
    PV accumulates ctxT[m, q] with v tiles stationary, softmax denominator
    via an all-ones [128,128] stationary matmul (gives the denominator
    pre-broadcast across partitions), then reciprocal_approx_fast and one
    multiply normalize the context.
  - matmul operands are bf16 (full PE rate); all accumulation is fp32 PSUM.
Causality: per 512-wide q block only the valid k tiles run; diagonal tiles
are masked post-exp with precomputed 0/1 masks; PV/denominator matmuls on
diagonal tiles only run over the valid column suffix.
"""

import math

import numpy as np

S = 4096
D = 2048
DT = D // 128           # 16 d-tiles
MLOC = 256              # head dims per core (2 heads x 128)
HLOC = 2                # heads per core
CH = 512                # s-chunk == attention q-block
NCORES = 8
SCALE = 1.0 / math.sqrt(128.0)


def _build(s_len, reps=1):
    import concourse.mybir as mybir
    import concourse.tile as tile
    from concourse import bacc

    f32 = mybir.dt.float32
    bf16 = mybir.dt.bfloat16
    AF = mybir.ActivationFunctionType

    n_qb = s_len // CH
    n_st = s_len // 128

    nc = bacc.Bacc()

    xTt = nc.dram_tensor("xTt", [128, s_len // CH, DT, CH], bf16, kind="ExternalInput")
    w_qTt = nc.dram_tensor("w_qTt", [128, DT, MLOC], bf16, kind="ExternalInput")
    w_kTt = nc.dram_tensor("w_kTt", [128, DT, MLOC], bf16, kind="ExternalInput")
    w_vTt = nc.dram_tensor("w_vTt", [128, DT, MLOC], bf16, kind="ExternalInput")
    w_oTt = nc.dram_tensor("w_oTt", [128, 2, D], bf16, kind="ExternalInput")
    cosT = nc.dram_tensor("cosT", [128, s_len], f32, kind="ExternalInput")
    sinTs = nc.dram_tensor("sinTs", [128, s_len], f32, kind="ExternalInput")
    # two diagonal-pair masks, each [128, 1024]: pair 0 = patterns (0,1),
    # pair 1 = patterns (2,3); pattern p is 1 where p*128 + k <= q.
    maskp = nc.dram_tensor("maskp", [128, 2048], bf16, kind="ExternalInput")
    outT = nc.dram_tensor("outT", [D, s_len], f32, kind="ExternalOutput")

    from contextlib import ExitStack

    with tile.TileContext(nc) as tc:
        with ExitStack() as ctx:
            consts = ctx.enter_context(tc.tile_pool(name="consts", bufs=1))
            wpool = ctx.enter_context(tc.tile_pool(name="wpool", bufs=1))
            kvpool = ctx.enter_context(tc.tile_pool(name="kvpool", bufs=1))
            xpool = ctx.enter_context(tc.tile_pool(name="xpool", bufs=2))
            qpool = ctx.enter_context(tc.tile_pool(name="qpool", bufs=2))
            ropepool = ctx.enter_context(tc.tile_pool(name="ropepool", bufs=2))
            tmppool = ctx.enter_context(tc.tile_pool(name="tmppool", bufs=3))
            epool = ctx.enter_context(tc.tile_pool(name="epool", bufs=6))
            ppair = ctx.enter_context(tc.tile_pool(name="ppair", bufs=4))
            dapool = ctx.enter_context(tc.tile_pool(name="dapool", bufs=2))
            ctxnpool = ctx.enter_context(tc.tile_pool(name="ctxnpool", bufs=2))
            rfpool = ctx.enter_context(tc.tile_pool(name="rfpool", bufs=2))
            obuf = ctx.enter_context(tc.tile_pool(name="obuf", bufs=8))
            pp1 = ctx.enter_context(tc.tile_pool(name="pp1", bufs=2, space="PSUM"))
            psc = ctx.enter_context(tc.tile_pool(name="psc", bufs=2, space="PSUM"))
            pam = ctx.enter_context(tc.tile_pool(name="pam", bufs=2, space="PSUM"))

            def body():
                _emit(
                    nc, tc, s_len, n_qb, n_st,
                    consts, wpool, kvpool, xpool, qpool, ropepool, tmppool,
                    epool, ppair, dapool, ctxnpool, rfpool, obuf, pp1, psc, pam,
                    xTt, w_qTt, w_kTt, w_vTt, w_oTt, cosT, sinTs, maskp, outT,
                    f32, bf16, AF,
                )

            if reps == 1:
                body()
            else:
                with tc.For_i(0, reps):
                    body()

    nc.finalize()
    return nc


def _emit(
    nc, tc, s_len, n_qb, n_st,
    consts, wpool, kvpool, xpool, qpool, ropepool, tmppool,
    epool, ppair, dapool, ctxnpool, rfpool, obuf, pp1, psc, pam,
    xTt, w_qTt, w_kTt, w_vTt, w_oTt, cosT, sinTs, maskp, outT,
    f32, bf16, AF,
):
            # ---- weights needed immediately ----
            w_q_sb = wpool.tile([128, DT, MLOC], bf16)
            for quarter in range(4):
                nc.scalar.dma_start(
                    out=w_q_sb[:, 4 * quarter : 4 * (quarter + 1)],
                    in_=w_qTt[:, 4 * quarter : 4 * (quarter + 1)],
                )
            w_k_sb = wpool.tile([128, DT, MLOC], bf16)
            nc.scalar.dma_start(out=w_k_sb, in_=w_kTt[:, :, :])
            # prefetch chunk 0 of x in d-tile quarters (so the first projection
            # matmuls can start after the first quarter lands) + rope tables
            # before the k/v weights
            xbig0 = xpool.tile([128, DT, CH], bf16, tag="xt", name="xt_pre0")
            for quarter in range(4):
                nc.sync.dma_start(
                    out=xbig0[:, 4 * quarter : 4 * (quarter + 1)],
                    in_=xTt[:, 0, 4 * quarter : 4 * (quarter + 1)],
                )
            cos0 = ropepool.tile([128, CH], f32, tag="cos", name="cos_pre0")
            nc.scalar.dma_start(out=cos0, in_=cosT[:, 0:CH])
            sin0 = ropepool.tile([128, CH], f32, tag="sin", name="sin_pre0")
            nc.scalar.dma_start(out=sin0, in_=sinTs[:, 0:CH])
            w_v_sb = wpool.tile([128, DT, MLOC], bf16)
            nc.scalar.dma_start(out=w_v_sb, in_=w_vTt[:, :, :])
            ones_sb = consts.tile([128, 128], bf16)
            nc.vector.memset(ones_sb, 1.0)

            # ---- persistent per-core tensors ----
            kT_sb = []
            for h in range(HLOC):
                kt = kvpool.tile([128, s_len], bf16, name=f"kT{h}")
                kT_sb.append(kt)
            v_sb = kvpool.tile([128, n_st * MLOC], bf16)

            w_o_sb = None
            mask_sb = None

            for i in range(n_qb):
                s0 = i * CH
                # ---- projections for s-chunk i ----
                if i == 0:
                    cos_t, sin_t, xbig = cos0, sin0, xbig0
                else:
                    cos_t = ropepool.tile([128, CH], f32, tag="cos", name=f"cos{i}")
                    nc.scalar.dma_start(out=cos_t, in_=cosT[:, s0 : s0 + CH])
                    sin_t = ropepool.tile([128, CH], f32, tag="sin", name=f"sin{i}")
                    nc.scalar.dma_start(out=sin_t, in_=sinTs[:, s0 : s0 + CH])
                    xbig = xpool.tile([128, DT, CH], bf16, tag="xt", name=f"xt{i}")
                    nc.sync.dma_start(out=xbig, in_=xTt[:, i])

                if i == 0:
                    # lower-priority loads, after the first chunk's stream
                    w_o_sb = wpool.tile([128, 2, D], bf16)
                    nc.scalar.dma_start(out=w_o_sb, in_=w_oTt[:, :, :])
                    mask_sb = consts.tile([128, 2048], bf16)
                    nc.scalar.dma_start(out=mask_sb, in_=maskp[:, :])

                q_t = []
                for h in range(HLOC):
                    for wname, w_sb in (("q", w_q_sb), ("k", w_k_sb)):
                        acc = pp1.tile(
                            [128, CH], f32, tag="pp1", name=f"acc_{wname}{h}_{i}"
                        )
                        for d in range(DT):
                            nc.tensor.matmul(
                                acc,
                                lhsT=w_sb[:, d, h * 128 : (h + 1) * 128],
                                rhs=xbig[:, d, :],
                                start=(d == 0),
                                stop=(d == DT - 1),
                            )
                        if wname == "q":
                            dst = qpool.tile(
                                [128, CH], bf16, tag=f"q{h}", name=f"q{h}_{i}"
                            )
                            q_t.append(dst)
                        else:
                            dst = kT_sb[h][:, s0 : s0 + CH]
                        # rope: dst = acc*cos + rot(acc)*sin_signed
                        t1 = tmppool.tile(
                            [128, CH], f32, tag="t1", name=f"t1_{wname}{h}_{i}"
                        )
                        nc.vector.tensor_mul(t1, acc, cos_t)
                        nc.vector.tensor_mul(dst[0:64], acc[64:128], sin_t[0:64])
                        nc.vector.tensor_mul(dst[64:128], acc[0:64], sin_t[64:128])
                        nc.vector.tensor_add(dst, dst, t1)

                for st in range(CH // 128):
                    vacc = pp1.tile([128, MLOC], f32, tag="pp1", name=f"vacc{st}_{i}")
                    for d in range(DT):
                        nc.tensor.matmul(
                            vacc,
                            lhsT=xbig[:, d, st * 128 : (st + 1) * 128],
                            rhs=w_v_sb[:, d, :],
                            start=(d == 0),
                            stop=(d == DT - 1),
                        )
                    gst = i * (CH // 128) + st
                    # Act engine (idle during the projection phase; Pool cannot
                    # read PSUM): keeps DVE free for rope/den work
                    nc.scalar.copy(v_sb[:, gst * MLOC : (gst + 1) * MLOC], vacc)

                # ---- attention for q-block i (k-tile pairs) ----
                nk = (i + 1) * (CH // 128)
                npair = nk // 2
                ctxn = []
                for h in range(HLOC):
                    ctx_ps = pam.tile([128, CH], f32, tag="am", name=f"ctx{h}_{i}")
                    # softmax denominator: fp32 accumulation of the bf16 exp
                    # pair-sums on DVE; one ones-matmul per (head, block) does
                    # the final cross-partition reduction
                    den_acc = dapool.tile([128, CH], f32, tag="da", name=f"da{h}_{i}")
                    ep_prev = None
                    for jp in range(npair):
                        p0 = 2 * jp - (CH // 128) * i  # diagonal pattern of half 0
                        fs = max(p0, 0) * 128  # valid suffix start of the pair
                        sc = psc.tile(
                            [128, 2, CH], f32, tag="sc", name=f"sc{h}_{i}_{jp}"
                        )
                        for half in range(2):
                            j = 2 * jp + half
                            # both halves stream [fs:CH] so the exp below never
                            # reads unwritten PSUM (half1's extra 128 cols get
                            # masked to zero after exp)
                            nc.tensor.matmul(
                                sc[:, half, fs:CH],
                                lhsT=kT_sb[h][:, j * 128 : (j + 1) * 128],
                                rhs=q_t[h][:, fs:CH],
                                start=True,
                                stop=True,
                            )
                        e = epool.tile([128, 2, CH], bf16, tag="e", name=f"e{h}_{i}_{jp}")
                        # exp only over columns the scores matmuls wrote: one
                        # flat call when the pair starts at 0, else one suffix
                        # call per half
                        if fs == 0:
                            nc.scalar.activation(
                                e.rearrange("p a b -> p (a b)"),
                                sc.rearrange("p a b -> p (a b)"),
                                AF.Exp,
                                scale=SCALE,
                            )
                        else:
                            for half in range(2):
                                nc.scalar.activation(
                                    e[:, half, fs:CH],
                                    sc[:, half, fs:CH],
                                    AF.Exp,
                                    scale=SCALE,
                                )
                        if p0 >= 0:  # diagonal pair: causal mask (patterns p0, p0+1)
                            moff = (p0 // 2) * 1024
                            if fs == 0:
                                nc.vector.tensor_mul(
                                    e.rearrange("p a b -> p (a b)"),
                                    e.rearrange("p a b -> p (a b)"),
                                    mask_sb[:, moff : moff + 1024],
                                )
                            else:
                                for half in range(2):
                                    nc.vector.tensor_mul(
                                        e[:, half, fs:CH],
                                        e[:, half, fs:CH],
                                        mask_sb[
                                            :,
                                            moff + half * CH + fs : moff
                                            + half * CH
                                            + CH,
                                        ],
                                    )
                        # pair-summed exp; non-diagonal pairs are further merged
                        # into quads so one ones-matmul covers 4 k-tiles
                        ep = ppair.tile([128, CH], bf16, tag="ep", name=f"ep{h}_{i}_{jp}")
                        nc.vector.tensor_add(
                            ep[:, fs:CH], e[:, 0, fs:CH], e[:, 1, fs:CH]
                        )
                        if p0 >= 0:  # diagonal pairs (always the last two)
                            if jp == npair - 1:  # p0 == 2, fs == 256
                                nc.vector.tensor_add(
                                    den_acc[:, fs:CH], den_acc[:, fs:CH], ep[:, fs:CH]
                                )
                            elif jp == 0:  # i == 0: first pair is diagonal, full
                                nc.vector.tensor_copy(den_acc, ep)
                            else:  # p0 == 0, full width
                                nc.vector.tensor_add(den_acc, den_acc, ep)
                        elif jp % 2 == 0:
                            ep_prev = ep
                        else:
                            if jp == 1:
                                nc.vector.tensor_add(den_acc, ep_prev, ep)
                            else:
                                eq = ppair.tile(
                                    [128, CH], bf16, tag="eq", name=f"eq{h}_{i}_{jp}"
                                )
                                nc.vector.tensor_add(eq, ep_prev, ep)
                                nc.vector.tensor_add(den_acc, den_acc, eq)
                        for half in range(2):
                            j = 2 * jp + half
                            p = j - (CH // 128) * i
                            qlo = max(p, 0) * 128  # valid column suffix start
                            nc.tensor.matmul(
                                ctx_ps[:, qlo:CH],
                                lhsT=v_sb[
                                    :, j * MLOC + h * 128 : j * MLOC + (h + 1) * 128
                                ],
                                rhs=e[:, half, qlo:CH],
                                start=(j == 0),
                                stop=(j == nk - 1),
                                skip_group_check=True,
                            )
                    den_bf = dapool.tile([128, CH], bf16, tag="db", name=f"db{h}_{i}")
                    nc.vector.tensor_copy(den_bf, den_acc)
                    den_ps = pam.tile([128, CH], f32, tag="am", name=f"den{h}_{i}")
                    nc.tensor.matmul(
                        den_ps,
                        lhsT=ones_sb,
                        rhs=den_bf,
                        start=True,
                        stop=True,
                        skip_group_check=True,
                    )
                    rf = rfpool.tile([128, CH], f32, tag="rf", name=f"rf{h}_{i}")
                    nc.vector.reciprocal_approx_fast(rf, den_ps)
                    cn = ctxnpool.tile([128, CH], bf16, tag=f"cn{h}", name=f"cn{h}_{i}")
                    nc.vector.tensor_mul(cn, ctx_ps, rf)
                    ctxn.append(cn)

                # ---- output projection for q-block i ----
                for t in range(DT):
                    # odd tiles borrow the (idle in this phase) scores pool so
                    # four PSUM banks rotate and the copy latency stays hidden
                    opool = pam if t % 2 == 0 else psc
                    o_ps = opool.tile(
                        [128, CH], f32,
                        tag="am" if t % 2 == 0 else "sc",
                        name=f"o{t}_{i}",
                    )
                    for ot in range(2):
                        nc.tensor.matmul(
                            o_ps,
                            lhsT=w_o_sb[:, ot, t * 128 : (t + 1) * 128],
                            rhs=ctxn[ot],
                            start=(ot == 0),
                            stop=(ot == 1),
                        )
                    o_sb = obuf.tile([128, CH], f32, tag="osb", name=f"osb{t}_{i}")
                    # alternate DVE/Act so neither engine's queue gates the
                    # PSUM buffer recycling (Pool cannot read PSUM)
                    if t % 2 == 1:
                        nc.scalar.copy(o_sb, o_ps)
                    else:
                        nc.vector.tensor_copy(o_sb, o_ps)
                    eng = nc.sync if t % 2 == 0 else nc.gpsimd
                    eng.dma_start(
                        out=outT[t * 128 : (t + 1) * 128, s0 : s0 + CH],
                        in_=o_sb,
                    )


def _host_inputs(x, w_q, w_k, w_v, w_o, s_len):
    """Host-side sharding / layout prep. Returns per-core input maps."""
    import ml_dtypes

    bf = ml_dtypes.bfloat16
    x2 = np.ascontiguousarray(x.reshape(s_len, D).astype(np.float32))
    xT = np.ascontiguousarray(x2.T.astype(bf))

    half = 64
    inv_freq = 1.0 / (10000.0 ** (np.arange(half, dtype=np.float32) / half))
    pos = np.arange(s_len, dtype=np.float32)
    ang = pos[:, None] * inv_freq[None, :]
    ang = np.concatenate([ang, ang], axis=1)  # [s, 128]
    cosT = np.ascontiguousarray(np.cos(ang).T.astype(np.float32))
    sinTs = np.ascontiguousarray(np.sin(ang).T.astype(np.float32))
    sinTs[:half] *= -1.0

    kk = np.arange(128)[:, None]
    qq = np.arange(512)[None, :]
    pats = [(kk + p * 128 <= qq).astype(bf) for p in range(4)]
    maskp = np.concatenate(
        [pats[0], pats[1], pats[2], pats[3]], axis=1
    )  # [128, 2048] = pairs (0,1),(2,3)

    # tiled layouts: [128, ...] partition-major so device DMAs are long
    # contiguous runs (descriptor-count-bound otherwise)
    xTt = np.ascontiguousarray(
        xT.reshape(16, 128, s_len // 512, 512).transpose(1, 2, 0, 3)
    )  # [128, n_ch, 16, 512]

    def wtile(wslice_T):  # [2048, 256] -> [128, 16, 256]
        return np.ascontiguousarray(wslice_T.reshape(16, 128, MLOC).transpose(1, 0, 2))

    in_maps = []
    for c in range(NCORES):
        rows = slice(MLOC * c, MLOC * (c + 1))
        w_oc = w_o[:, rows].T.astype(bf)  # [256, 2048]
        in_maps.append(
            {
                "xTt": xTt,
                "w_qTt": wtile(w_q[rows].T.astype(bf)),
                "w_kTt": wtile(w_k[rows].T.astype(bf)),
                "w_vTt": wtile(w_v[rows].T.astype(bf)),
                "w_oTt": np.ascontiguousarray(
                    w_oc.reshape(2, 128, D).transpose(1, 0, 2)
                ),
                "cosT": cosT,
                "sinTs": sinTs,
                "maskp": maskp,
            }
        )
    return in_maps


_NC_CACHE = {}


def kernel(x, w_q, w_k, w_v, w_o):
    from concourse.bass_utils import run_bass_kernel_spmd

    s_len = x.shape[1]
    if s_len not in _NC_CACHE:
        _NC_CACHE[s_len] = _build(s_len)
    nc = _NC_CACHE[s_len]

    in_maps = _host_inputs(
        np.asarray(x), np.asarray(w_q), np.asarray(w_k), np.asarray(w_v),
        np.asarray(w_o), s_len,
    )
    res = run_bass_kernel_spmd(nc, in_maps, core_ids=list(range(NCORES)))
    acc = np.zeros((D, s_len), dtype=np.float32)
    for r in res.results:
        acc += r["outT"]
    return np.ascontiguousarray(acc.T)[None].astype(np.float32)

